# revision 11
# baseline (speedup 1.0000x reference)
"""Trainium2 Bass kernel for nn_ContrastiveLearningWrapper.

Computes: z = normalize(x @ W.T + b); predictions = sigmoid(z @ z.T / T);
BCE loss split into positive (same-label, off-diag) / negative means.

Strategy (8 NeuronCores, data-parallel over rows):
  - Each core gets its 512-row shard of x, computes its normalized
    projection shard zT_c [128, 512] (columns pre-scaled by sqrt(1/T) so the
    Gram matrix is directly the logit s = cos/T).
  - AllGather of the normalized projections -> zT_full [128, 4096] on every
    core.
  - Each core computes its [512, 4096] slab of predictions = sigmoid(G) and
    accumulates sum(ln(1 - p)) over all its elements via the ACT accum_out
    port (q = 1-p on DVE, Ln on ACT) -> a2 partial sums.
  - Host combines: the dense negative-sum comes from the device a2 totals;
    the sparse same-label corrections (~1.6% of pairs) are computed exactly
    on host from the gathered z shards; final scalar assembly in fp64.
"""

import math

import numpy as np

import concourse.bacc as bacc
import concourse.bass as bass
import concourse.mybir as mybir
import concourse.tile as tile
from concourse.bass_utils import run_bass_kernel_spmd
from concourse.masks import make_identity

FP32 = mybir.dt.float32

N = 4096  # total rows
D = 512  # input feature dim
P = 128  # projection dim (= partition count)
NCORES = 8
NB = N // NCORES  # rows per core (512)
RT = NB // P  # row tiles per core (4)
KD = D // P  # contraction chunks for projection (4)
TEMPERATURE = 0.1
# zT columns are scaled by sqrt(1/T) so G = zn.T@zn equals cos/T directly.
HALF_LN_INV_T = 0.5 * math.log(1.0 / TEMPERATURE)
NLAB = 64
NL_WEIGHT = 0.5


def build_kernel():
    nc = bacc.Bacc(
        "TRN2",
        target_bir_lowering=False,
        debug=False,
        enable_asserts=False,
        num_devices=NCORES,
    )

    xc = nc.dram_tensor("xc", [NB, D], FP32, kind="ExternalInput").ap()
    w = nc.dram_tensor("w", [P, D], FP32, kind="ExternalInput").ap()
    bvec = nc.dram_tensor("bvec", [P, 1], FP32, kind="ExternalInput").ap()

    pred_out = nc.dram_tensor("pred_out", [NB, N], FP32, kind="ExternalOutput").ap()
    zc_out = nc.dram_tensor("zc_out", [P, NB], FP32, kind="ExternalOutput").ap()
    a2_out = nc.dram_tensor("a2_out", [P, 2 * RT], FP32, kind="ExternalOutput").ap()

    with tile.TileContext(nc) as tc:
        _kernel_body(tc, xc, w, bvec, pred_out, zc_out, a2_out)

    nc.compile()
    return nc


def _kernel_body(tc, xc, w, bvec, pred_out, zc_out, a2_out):
    nc = tc.nc
    AF = mybir.ActivationFunctionType
    ALU = mybir.AluOpType

    with (
        tc.tile_pool(name="consts", bufs=1) as consts,
        tc.tile_pool(name="work", bufs=1) as work,
    ):
        identity = consts.tile([P, P], FP32)
        make_identity(nc, identity)
        ones_col = consts.tile([P, 1], FP32)
        nc.vector.memset(ones_col, 1.0)
        ones_row = consts.tile([1, P], FP32)
        nc.vector.memset(ones_row, 1.0)

        # x shard first — the transposes (critical path) wait on it
        xc_sb = consts.tile([P, RT * D], FP32)
        nc.sync.dma_start(
            xc_sb[:].rearrange("p (i d) -> p i d", d=D),
            xc.rearrange("(i p) d -> p i d", p=P),
        )
        w_sb = consts.tile([P, D], FP32)
        nc.sync.dma_start(w_sb[:], w)
        b_sb = consts.tile([P, 1], FP32)
        nc.sync.dma_start(b_sb[:], bvec)

        with tc.tile_pool(name="psum0", bufs=1, space="PSUM") as psum0:
            # --- W.T chunks: wT_sb[:, k*128:(k+1)*128] = W[:, k-chunk].T
            pw = psum0.tile([P, 4 * P], FP32)
            for k in range(KD):
                nc.tensor.transpose(
                    pw[:, k * P : (k + 1) * P],
                    w_sb[:, k * P : (k + 1) * P],
                    identity,
                )
            wT_sb = consts.tile([P, 4 * P], FP32)
            nc.vector.tensor_copy(wT_sb[:], pw[:])

            # --- x_c.T: xT_sb[:, k*NB + r] = x_c[r, k*128 + <part>]
            xT_sb = consts.tile([P, KD * NB], FP32)
            for k in range(KD):
                px = psum0.tile([P, NB], FP32, tag="px", bufs=4)
                for i in range(RT):
                    nc.tensor.transpose(
                        px[:, i * P : (i + 1) * P],
                        xc_sb[:, i * D + k * P : i * D + (k + 1) * P],
                        identity,
                    )
                nc.vector.tensor_copy(xT_sb[:, k * NB : (k + 1) * NB], px[:])

            # --- projection: zproj[p, r] = sum_d W[p, d] * x_c[r, d]
            zproj = psum0.tile([P, NB], FP32)
            for k in range(KD):
                nc.tensor.matmul(
                    zproj[:],
                    wT_sb[:, k * P : (k + 1) * P],
                    xT_sb[:, k * NB : (k + 1) * NB],
                    start=(k == 0),
                    stop=(k == KD - 1),
                )

            # --- normalization: rn[r] = sqrt(1/T) / ||z_r||
            zsq = work.tile([P, NB], FP32)
            nc.scalar.activation(zsq[:], zproj[:], AF.Square, bias=b_sb[:], scale=1.0)
            n2p = psum0.tile([1, NB], FP32)
            nc.tensor.matmul(n2p[:], ones_col[:], zsq[:], start=True, stop=True)
            lnp = work.tile([1, NB], FP32)
            nc.scalar.activation(lnp[:], n2p[:], AF.Ln)
            rn = work.tile([1, NB], FP32)
            half_ln_inv_t = work.tile([1, 1], FP32)
            nc.vector.memset(half_ln_inv_t, HALF_LN_INV_T)
            nc.scalar.activation(
                rn[:], lnp[:], AF.Exp, bias=half_ln_inv_t[:], scale=-0.5
            )
            # broadcast rn across partitions via K=1 matmul
            rnb_p = psum0.tile([P, NB], FP32)
            nc.tensor.matmul(rnb_p[:], ones_row[:], rn[:], start=True, stop=True)
            rnb = work.tile([P, NB], FP32)
            nc.vector.tensor_copy(rnb[:], rnb_p[:])

            # zn[p, r] = (zproj[p, r] + b[p]) * rn[r]
            zn = consts.tile([P, NB], FP32)
            nc.vector.scalar_tensor_tensor(
                out=zn[:],
                in0=zproj[:],
                scalar=b_sb[:],
                in1=rnb[:],
                op0=ALU.add,
                op1=ALU.mult,
            )

        # --- AllGather across the 8 cores (issued first: it gates everything)
        with tc.tile_pool(name="dram", bufs=1, space="DRAM") as dram:
            bounce_in = dram.tile([P, NB], FP32)
            bounce_out = dram.tile([NCORES * P, NB], FP32, addr_space="Shared")
            nc.sync.dma_start(bounce_in[:], zn[:])
            nc.gpsimd.collective_compute(
                "AllGather",
                ALU.bypass,
                replica_groups=[list(range(NCORES))],
                ins=[bounce_in[:].opt()],
                outs=[bounce_out[:].opt()],
            )
            # chunked readback: the first G matmuls start after one 256KB
            # slice instead of waiting for the whole 2MB gather readback
            zT_full = consts.tile([P, N], FP32)
            for r in range(NCORES):
                nc.sync.dma_start(
                    zT_full[:, r * NB : (r + 1) * NB],
                    bounce_out[r * P : (r + 1) * P, :],
                )
        # shard to host (off the critical path)
        nc.sync.dma_start(zc_out, zn[:])

        # --- main pass: G = zn_i.T @ zT_full, predictions + loss partials
        HW = N // 2  # half-row width (2048), 4 PSUM banks
        QW = HW // 4  # quad-product width for the Ln pass (512)
        a2sb = consts.tile([P, 2 * RT], FP32)
        # prod = product of 4 q values (q = 1-p): quarters ACT work in Ln pass
        prod_sb = consts.tile([P, 2 * RT * QW], FP32)

        sig_insts = []
        ln_insts = []
        with (
            tc.tile_pool(name="psumG", bufs=2, space="PSUM") as psumg,
            tc.tile_pool(name="outbuf", bufs=2) as outbuf,
            tc.tile_pool(name="qbuf", bufs=2) as qbuf,
        ):
            # Phase A: sigmoid (one ACT table set), predictions out, q products
            for i in range(RT):
                lhsT_i = zn[:, i * P : (i + 1) * P]
                pbuf = outbuf.tile([P, N], FP32, tag="pred", bufs=2)
                for h in range(2):
                    g = psumg.tile([P, HW], FP32, tag="g", bufs=2)
                    for qd in range(4):
                        jc = h * 4 + qd
                        nc.tensor.matmul(
                            g[:, qd * 512 : (qd + 1) * 512],
                            lhsT_i,
                            zT_full[:, jc * 512 : (jc + 1) * 512],
                            start=True,
                            stop=True,
                        )
                    ph = pbuf[:, h * HW : (h + 1) * HW]
                    sig_insts.append(nc.scalar.activation(ph, g[:], AF.Sigmoid))
                    slot = 2 * i + h
                    qt = qbuf.tile([P, HW], FP32, tag="q", bufs=2)
                    nc.vector.tensor_scalar(
                        qt[:], ph, -1.0, 1.0, ALU.mult, ALU.add
                    )
                    q2 = qbuf.tile([P, HW // 2], FP32, tag="q2", bufs=2)
                    nc.vector.tensor_tensor(
                        q2[:], qt[:, : HW // 2], qt[:, HW // 2 :], ALU.mult
                    )
                    nc.vector.tensor_tensor(
                        prod_sb[:, slot * QW : (slot + 1) * QW],
                        q2[:, :QW],
                        q2[:, QW:],
                        ALU.mult,
                    )
                nc.sync.dma_start(pred_out[i * P : (i + 1) * P, :], pbuf[:])

            # Phase B: ln(prod) with fused row-sum accumulation.
            # ln(q_a * q_b) = ln q_a + ln q_b, so summing over products is the
            # same A2 total with half the ACT elements.
            for slot in range(2 * RT):
                lnbuf = outbuf.tile([P, QW], FP32, tag="lnb", bufs=2)
                ln_insts.append(
                    nc.scalar.activation(
                        lnbuf[:],
                        prod_sb[:, slot * QW : (slot + 1) * QW],
                        AF.Ln,
                        accum_out=a2sb[:, slot : slot + 1],
                    )
                )

        # Keep the ACT stream phase-ordered: every Ln after the last sigmoid,
        # so the sigmoid/natural_log table sets each load exactly once.
        last_sig = sig_insts[-1]
        for li in ln_insts:
            tile.add_dep_helper(li.ins, last_sig.ins, sync=False,
                                reason="batch ACT table sets")

        nc.sync.dma_start(a2_out, a2sb[:])


_NC_CACHE = None


def _get_nc():
    global _NC_CACHE
    if _NC_CACHE is None:
        _NC_CACHE = build_kernel()
    return _NC_CACHE


def kernel(outputs, targets, W, b):
    x = np.ascontiguousarray(np.asarray(outputs)[:, 0, :], dtype=np.float32)
    t = np.asarray(targets).reshape(-1).astype(np.int64)
    Wf = np.ascontiguousarray(np.asarray(W), dtype=np.float32)
    bf = np.ascontiguousarray(np.asarray(b), dtype=np.float32).reshape(P, 1)

    nc = _get_nc()
    in_maps = [
        {"xc": x[c * NB : (c + 1) * NB], "w": Wf, "bvec": bf} for c in range(NCORES)
    ]
    res = run_bass_kernel_spmd(nc, in_maps, core_ids=list(range(NCORES)))
    results = res.results

    predictions = np.concatenate([r["pred_out"] for r in results], axis=0)
    # zc_out is [P, NB] (projection-dim major); rows of z are its columns
    zn = np.concatenate([r["zc_out"].T for r in results], axis=0).astype(np.float64)
    # device sum of ln(1 - p) over all pairs
    a2 = sum(float(r["a2_out"].astype(np.float64).sum()) for r in results)

    # host corrections over same-label pairs (sparse, exact in fp64)
    c2 = 0.0  # sum_eq ln(1 - sigmoid(s)) = sum_eq -softplus(s)
    pos_num = 0.0  # sum_pos softplus(-s)
    pos_cnt = 0
    for k in range(NLAB):
        idx = np.flatnonzero(t == k)
        nk = idx.size
        if nk == 0:
            continue
        Zk = zn[idx]
        S = Zk @ Zk.T
        c2 += -np.logaddexp(0.0, S).sum()
        sp_neg = np.logaddexp(0.0, -S)
        pos_num += sp_neg.sum() - np.trace(sp_neg)
        pos_cnt += nk * (nk - 1)

    neg_cnt = N * N - N - pos_cnt
    neg_num = -(a2 - c2)  # sum over different-label pairs of -ln(1 - p)
    loss = np.float32(pos_num / pos_cnt + NL_WEIGHT * neg_num / neg_cnt)
    return loss, predictions
